# revision 14
# baseline (speedup 1.0000x reference)
"""Cross-attention fusion block on 8 trn2 NeuronCores.

Sharding: data-parallel over the query sequence (S=4096 -> 512 rows/core).
K/V projections are computed redundantly on every core (cheap vs attention).
Everything runs in channel-major ("transposed") layout [C, S] so that no
on-chip transposes are needed anywhere:
  inputs  lidar/image [1,C,H,W] -> [C, S]   (natural memory layout)
  output  [C, S] -> [1, C, H, W]            (natural memory layout)

Per-core pipeline (q = 512 query rows of this core), fp32r matmuls:
  qT = Wq^T @ xT (+bq)            [256, 512]
  kT = Wk^T @ y                   [256, 4096]   (bk dropped: softmax-invariant)
  v2 = y^T @ Wv, stored per-head as [V_h | 1]   (bv folded in after softmax)
  per 4-head group, per 128-row key chunk (ST tiles of 2 heads, row-packed
  4-way on the PE):
    ST[k,q]   = K_h chunk @ Q_h^T
    AT        = exp(ST / sqrt(32))              (ACT, no max subtraction)
    AVCS_h   += [V_h | 1]^T-chunk @ AT          (rows 0-31 attn@v, row 32
                                                 softmax denominator; one
                                                 accumulation chain per bank)
  attn_h = AV_h * (1/CS_h) + bv    (stage copy + DMA partition shifts; CS
                                    rows bounce through DRAM for broadcast)
  o = Wo^T @ attn (+bo); r = qT + o; z = LN(r)  (stats via ones-matmuls)
  h1 = relu(W1^T @ z + bf1); h2 = W2^T @ h1 + bf2; out = LN(z + h2)
"""

import sys

for _p in ("/opt/trn_rl_repo", "/opt/pypackages"):
    if _p not in sys.path:
        sys.path.append(_p)

import numpy as np

import concourse.bass as bass
import concourse.bacc as bacc
import concourse.tile as tile
from concourse import mybir
from concourse.bass_utils import run_bass_kernel_spmd

F32 = mybir.dt.float32
F32R = mybir.dt.float32r
AFT = mybir.ActivationFunctionType
ALU = mybir.AluOpType

P = 128           # SBUF partitions
C = 256           # channels
S = 4096          # sequence (64*64)
NCORES = 8
SH = S // NCORES  # 512 query rows per core
NH = 8            # heads
HD = 32           # head dim
HD1 = HD + 1      # V block plus the ones column for the colsum
F = 4 * C         # FFN hidden = 1024
NKC = C // P      # 2 channel chunks
NFC = F // P      # 8 ffn chunks
NSC = S // P      # 32 key chunks
EPS = 1e-5
INV_SQRT_HD = 1.0 / float(np.sqrt(HD))
INV_C = 1.0 / C


def build_bass():
    nc = bacc.Bacc()

    xT = nc.declare_dram_parameter("xT", [C, SH], F32R, isOutput=False)
    y = nc.declare_dram_parameter("y", [C, S], F32R, isOutput=False)
    wq = nc.declare_dram_parameter("wq", [C, C], F32R, isOutput=False)
    wk = nc.declare_dram_parameter("wk", [C, C], F32R, isOutput=False)
    wv = nc.declare_dram_parameter("wv", [C, C], F32R, isOutput=False)
    wo = nc.declare_dram_parameter("wo", [C, C], F32R, isOutput=False)
    w1 = nc.declare_dram_parameter("w1", [C, F], F32R, isOutput=False)
    w2 = nc.declare_dram_parameter("w2", [F, C], F32R, isOutput=False)
    ones32 = nc.declare_dram_parameter("ones32", [P, 1], F32R, isOutput=False)
    bq = nc.declare_dram_parameter("bq", [C], F32, isOutput=False)
    bv = nc.declare_dram_parameter("bv", [C], F32, isOutput=False)
    bo = nc.declare_dram_parameter("bo", [C], F32, isOutput=False)
    bf1 = nc.declare_dram_parameter("bf1", [F], F32, isOutput=False)
    bf2 = nc.declare_dram_parameter("bf2", [C], F32, isOutput=False)
    g1 = nc.declare_dram_parameter("g1", [C], F32, isOutput=False)
    b1 = nc.declare_dram_parameter("b1", [C], F32, isOutput=False)
    g2 = nc.declare_dram_parameter("g2", [C], F32, isOutput=False)
    b2 = nc.declare_dram_parameter("b2", [C], F32, isOutput=False)
    out = nc.declare_dram_parameter("out", [C, SH], F32, isOutput=True)

    cs_scr = nc.dram_tensor("cs_scr", [2, 4, SH], F32)
    rec_scr = nc.dram_tensor("rec_scr", [2, 4, SH], F32)

    with tile.TileContext(nc) as tc:
        _emit(tc, xT, y, wq, wk, wv, wo, w1, w2, ones32,
              bq, bv, bo, bf1, bf2, g1, b1, g2, b2, out, cs_scr, rec_scr)
    if not nc.is_finalized():
        nc.finalize()
    return nc


def _emit(tc, xT, y, wq, wk, wv, wo, w1, w2, ones32,
          bq, bv, bo, bf1, bf2, g1, b1, g2, b2, out, cs_scr, rec_scr):
    nc = tc.nc

    import contextlib
    stack = contextlib.ExitStack()
    with stack:
        consts = stack.enter_context(tc.tile_pool(name="consts", bufs=1))
        big = stack.enter_context(tc.tile_pool(name="big", bufs=1))

        # ---------------- constants / weights into SBUF ----------------
        y_sb = big.tile([P, NKC, S], F32R)         # y[ch, s]; ch = kc*128 + p
        nc.sync.dma_start(out=y_sb, in_=y.rearrange("(kc p) s -> p kc s", p=P))
        xT_sb = big.tile([P, NKC, SH], F32R)
        nc.sync.dma_start(out=xT_sb, in_=xT.rearrange("(kc p) s -> p kc s", p=P))

        wq_sb = consts.tile([P, NKC, C], F32R)
        nc.sync.dma_start(out=wq_sb, in_=wq.rearrange("(kc p) m -> p kc m", p=P))
        wk_sb = consts.tile([P, NKC, C], F32R)
        nc.sync.dma_start(out=wk_sb, in_=wk.rearrange("(kc p) m -> p kc m", p=P))
        wv_sb = consts.tile([P, NKC, C], F32R)
        nc.sync.dma_start(out=wv_sb, in_=wv.rearrange("(kc p) m -> p kc m", p=P))
        wo_sb = consts.tile([P, NKC, C], F32R)
        nc.sync.dma_start(out=wo_sb, in_=wo.rearrange("(kc p) m -> p kc m", p=P))
        w1_sb = consts.tile([P, NKC, F], F32R)
        nc.sync.dma_start(out=w1_sb, in_=w1.rearrange("(kc p) m -> p kc m", p=P))
        w2_sb = consts.tile([P, NFC, C], F32R)
        nc.sync.dma_start(out=w2_sb, in_=w2.rearrange("(kc p) m -> p kc m", p=P))

        bq_sb = consts.tile([P, NKC], F32)
        nc.sync.dma_start(out=bq_sb, in_=bq.rearrange("(kc p) -> p kc", p=P))
        bo_sb = consts.tile([P, NKC], F32)
        nc.sync.dma_start(out=bo_sb, in_=bo.rearrange("(kc p) -> p kc", p=P))
        bf1_sb = consts.tile([P, NFC], F32)
        nc.sync.dma_start(out=bf1_sb, in_=bf1.rearrange("(kc p) -> p kc", p=P))
        bf2_sb = consts.tile([P, NKC], F32)
        nc.sync.dma_start(out=bf2_sb, in_=bf2.rearrange("(kc p) -> p kc", p=P))
        g1_sb = consts.tile([P, NKC], F32)
        nc.sync.dma_start(out=g1_sb, in_=g1.rearrange("(kc p) -> p kc", p=P))
        b1_sb = consts.tile([P, NKC], F32)
        nc.sync.dma_start(out=b1_sb, in_=b1.rearrange("(kc p) -> p kc", p=P))
        g2_sb = consts.tile([P, NKC], F32)
        nc.sync.dma_start(out=g2_sb, in_=g2.rearrange("(kc p) -> p kc", p=P))
        b2_sb = consts.tile([P, NKC], F32)
        nc.sync.dma_start(out=b2_sb, in_=b2.rearrange("(kc p) -> p kc", p=P))
        bv_sb = consts.tile([P, NKC], F32)
        nc.sync.dma_start(out=bv_sb, in_=bv.rearrange("(kc p) -> p kc", p=P))

        ones_mean = consts.tile([P, 1], F32)    # LN-stats lhsT (pre-scaled 1/C)
        nc.vector.memset(ones_mean, INV_C)
        ones_rep = consts.tile([1, P], F32)     # K=1 row-replication lhsT
        nc.vector.memset(ones_rep, 1.0)
        eps_sb = consts.tile([P, 1], F32)
        nc.vector.memset(eps_sb, EPS)

        # persistent activations
        qT_sb = big.tile([P, NKC, SH], F32R)    # q^T  (with bq)
        kT_sb = big.tile([P, NKC, S], F32R)     # k^T  (no bk; softmax-invariant)
        v2_sb = big.tile([P, NSC, NH, HD1], F32R)  # per head [V_h | 1]
        attn_sb = big.tile([P, NKC, SH], F32R)  # (attn @ v)^T + bv
        z_sb = big.tile([P, NKC, SH], F32R)     # LN1 output
        h1_sb = big.tile([P, NFC, SH], F32R)    # relu(ffn1)
        out_sb = big.tile([P, NKC, SH], F32)    # final
        r_sb = big.tile([P, NKC, SH], F32)      # residual sums (LN inputs)

        # ones column of v2 (memset cannot write f32r -> DMA broadcast)
        ones_ap = ones32[:]
        ones_col = bass.AP(
            tensor=ones_ap.tensor, offset=ones_ap.offset,
            ap=[ones_ap.ap[0], [0, NSC * NH]])
        nc.sync.dma_start(out=v2_sb[:, :, :, HD:HD1].squeeze(),
                          in_=ones_col)

        # ---------------- preamble: qT, kT, v projections ----------------
        with tc.tile_pool(name="pre_k", bufs=3, space="PSUM") as pre_k, \
             tc.tile_pool(name="pre_v", bufs=3, space="PSUM") as pre_v:
            # q^T[c',q] = sum_ch Wq[ch,c'] xT[ch,q]
            for mc in range(NKC):
                ps = pre_k.tile([P, SH], F32, tag="ps_k")
                for kc in range(NKC):
                    nc.tensor.matmul(
                        ps, wq_sb[:, kc, mc * P:(mc + 1) * P],
                        xT_sb[:, kc, :],
                        start=(kc == 0), stop=(kc == NKC - 1))
                nc.scalar.activation(out=qT_sb[:, mc, :], in_=ps,
                                     func=AFT.Identity,
                                     bias=bq_sb[:, mc:mc + 1])
            # k^T[c',s] = sum_ch Wk[ch,c'] y[ch,s]   in 512-col blocks
            for sb in range(S // 512):
                for mc in range(NKC):
                    ps = pre_k.tile([P, 512], F32, tag="ps_k")
                    for kc in range(NKC):
                        nc.tensor.matmul(
                            ps, wk_sb[:, kc, mc * P:(mc + 1) * P],
                            y_sb[:, kc, sb * 512:(sb + 1) * 512],
                            start=(kc == 0), stop=(kc == NKC - 1))
                    nc.scalar.activation(
                        out=kT_sb[:, mc, sb * 512:(sb + 1) * 512],
                        in_=ps, func=AFT.Copy)
            # v[s,c'] = sum_ch y[ch,s] Wv[ch,c']    per 128-row s chunk
            for ck in range(NSC):
                ps = pre_v.tile([P, C], F32, tag="ps_v")
                for kc in range(NKC):
                    nc.tensor.matmul(
                        ps, y_sb[:, kc, ck * P:(ck + 1) * P],
                        wv_sb[:, kc, :],
                        start=(kc == 0), stop=(kc == NKC - 1))
                # scatter the 8 per-head blocks into the [V_h | 1] layout
                dst = v2_sb[:, ck, :, 0:HD]
                nc.vector.tensor_copy(dst, ps.rearrange("p (h d) -> p h d",
                                                        d=HD))

        # ---------------- attention ----------------
        with tc.tile_pool(name="st", bufs=2, space="PSUM") as st_pool, \
             tc.tile_pool(name="avcs", bufs=4, space="PSUM") as avcs_pool, \
             tc.tile_pool(name="at", bufs=4) as at_pool, \
             tc.tile_pool(name="nrm", bufs=2) as nrm_pool:
            for grp in range(2):
                avcs = [avcs_pool.tile([HD1, SH], F32, tag="avcs",
                                       name=f"avcs_g{grp}_{j}")
                        for j in range(4)]
                for ck in range(NSC):
                    for pair in range(2):
                        st = st_pool.tile([P, 2, SH], F32, tag="st")
                        for j in range(2):
                            h = 4 * grp + 2 * pair + j
                            po = HD * (h % 4)
                            nc.tensor.matmul(
                                st[:, j, :],
                                kT_sb[po:po + HD, grp, ck * P:(ck + 1) * P],
                                qT_sb[po:po + HD, grp, :],
                                start=True, stop=True,
                                tile_position=(po, 0))
                        at = at_pool.tile([P, 2, SH], F32R, tag="at")
                        nc.scalar.activation(out=at, in_=st, func=AFT.Exp,
                                             scale=INV_SQRT_HD)
                        for j in range(2):
                            h = 4 * grp + 2 * pair + j
                            nc.tensor.matmul(
                                avcs[2 * pair + j],
                                v2_sb[:, ck, h, :],
                                at[:, j, :],
                                start=(ck == 0), stop=(ck == NSC - 1))
                # normalize: attn_h = av_h / cs_h (+ bv later, whole group)
                av_all = nrm_pool.tile([P, SH], F32, tag="av_all")
                for j in range(4):
                    stg = nrm_pool.tile([HD1, SH], F32, tag="stg")
                    nc.scalar.activation(out=stg, in_=avcs[j], func=AFT.Copy)
                    nc.sync.dma_start(out=av_all[HD * j:HD * (j + 1), :],
                                      in_=stg[0:HD, :])
                    nc.sync.dma_start(out=cs_scr[grp, j, :],
                                      in_=stg[HD:HD1, :])
                cs4 = nrm_pool.tile([4, SH], F32, tag="cs4")
                nc.sync.dma_start(out=cs4, in_=cs_scr[grp])
                rec4 = nrm_pool.tile([4, SH], F32, tag="rec4")
                scr4 = nrm_pool.tile([4, SH], F32, tag="scr4")
                nc.vector.reciprocal_approx_accurate(out=rec4, in_=cs4,
                                                     scratch=scr4)
                nc.sync.dma_start(out=rec_scr[grp], in_=rec4)
                rec_all = nrm_pool.tile([P, SH], F32, tag="rec_all")
                for j in range(4):
                    row = rec_scr[grp, j, :]
                    rep = bass.AP(tensor=row.tensor, offset=row.offset,
                                  ap=[[0, HD]] + row.ap)
                    nc.sync.dma_start(out=rec_all[HD * j:HD * (j + 1), :],
                                      in_=rep)
                tmp = nrm_pool.tile([P, SH], F32, tag="tmp")
                nc.vector.tensor_mul(tmp, av_all, rec_all)
                nc.vector.tensor_scalar_add(out=attn_sb[:, grp, :], in0=tmp,
                                            scalar1=bv_sb[:, grp:grp + 1])

        # ---------------- tail: out-proj, LN1, FFN, LN2 ----------------
        with tc.tile_pool(name="mm", bufs=3, space="PSUM") as mm_pool, \
             tc.tile_pool(name="stat", bufs=1, space="PSUM") as stat_pool, \
             tc.tile_pool(name="rep", bufs=1, space="PSUM") as rep_pool, \
             tc.tile_pool(name="tl", bufs=2) as tl_pool, \
             tc.tile_pool(name="tr", bufs=1) as tr_pool:

            def layer_norm(x3, gamma, beta, out3):
                """out3 = LN(x3) over the channel axis (2 chunks of 128)."""
                mu_ps = stat_pool.tile([1, SH], F32, tag="mu")
                e2_ps = stat_pool.tile([1, SH], F32, tag="e2")
                for kc in range(NKC):
                    nc.tensor.matmul(mu_ps, ones_mean, x3[:, kc, :],
                                     start=(kc == 0), stop=(kc == NKC - 1))
                for kc in range(NKC):
                    sq = tl_pool.tile([P, SH], F32, tag="sq")
                    nc.vector.tensor_mul(sq, x3[:, kc, :], x3[:, kc, :])
                    nc.tensor.matmul(e2_ps, ones_mean, sq,
                                     start=(kc == 0), stop=(kc == NKC - 1))
                mu_row = tr_pool.tile([1, SH], F32, tag="mu_row")
                nc.scalar.activation(out=mu_row, in_=mu_ps, func=AFT.Copy)
                mu2_row = tr_pool.tile([1, SH], F32, tag="mu2_row")
                nc.vector.tensor_mul(mu2_row, mu_row, mu_row)
                var_row = tr_pool.tile([1, SH], F32, tag="var_row")
                nc.vector.tensor_sub(var_row, e2_ps, mu2_row)
                std_row = tr_pool.tile([1, SH], F32, tag="std_row")
                nc.scalar.activation(out=std_row, in_=var_row, func=AFT.Sqrt,
                                     bias=eps_sb[:1, :])
                rstd_row = tr_pool.tile([1, SH], F32, tag="rstd_row")
                scr_row = tr_pool.tile([1, SH], F32, tag="mu2_row")
                nc.vector.reciprocal_approx_accurate(out=rstd_row, in_=std_row,
                                                     scratch=scr_row)
                mu_rep = rep_pool.tile([P, SH], F32, tag="mu_rep")
                nc.tensor.matmul(mu_rep, ones_rep, mu_row,
                                 start=True, stop=True)
                rstd_rep = rep_pool.tile([P, SH], F32, tag="rstd_rep")
                nc.tensor.matmul(rstd_rep, ones_rep, rstd_row,
                                 start=True, stop=True)
                for kc in range(NKC):
                    t = tl_pool.tile([P, SH], F32, tag="t")
                    nc.vector.tensor_sub(t, x3[:, kc, :], mu_rep)
                    t2 = tl_pool.tile([P, SH], F32, tag="t2")
                    nc.vector.tensor_mul(t2, t, rstd_rep)
                    nc.vector.tensor_scalar(
                        out=out3[:, kc, :], in0=t2,
                        scalar1=gamma[:, kc:kc + 1], scalar2=beta[:, kc:kc + 1],
                        op0=ALU.mult, op1=ALU.add)

            # out-projection + residual (r = qT + Wo^T attn + bo)
            for mc in range(NKC):
                ps = mm_pool.tile([P, SH], F32, tag="mm")
                for kc in range(NKC):
                    nc.tensor.matmul(
                        ps, wo_sb[:, kc, mc * P:(mc + 1) * P],
                        attn_sb[:, kc, :],
                        start=(kc == 0), stop=(kc == NKC - 1))
                o_t = tl_pool.tile([P, SH], F32, tag="o_t")
                nc.scalar.activation(out=o_t, in_=ps, func=AFT.Identity,
                                     bias=bo_sb[:, mc:mc + 1])
                nc.vector.tensor_add(r_sb[:, mc, :], qT_sb[:, mc, :], o_t)

            layer_norm(r_sb, g1_sb, b1_sb, z_sb)

            # FFN1 + relu
            for mf in range(NFC):
                ps = mm_pool.tile([P, SH], F32, tag="mm")
                for kc in range(NKC):
                    nc.tensor.matmul(
                        ps, w1_sb[:, kc, mf * P:(mf + 1) * P],
                        z_sb[:, kc, :],
                        start=(kc == 0), stop=(kc == NKC - 1))
                nc.vector.tensor_scalar(
                    out=h1_sb[:, mf, :], in0=ps,
                    scalar1=bf1_sb[:, mf:mf + 1], scalar2=0.0,
                    op0=ALU.add, op1=ALU.max)
            # FFN2 + bias + residual
            for mc in range(NKC):
                ps = mm_pool.tile([P, SH], F32, tag="mm")
                for kf in range(NFC):
                    nc.tensor.matmul(
                        ps, w2_sb[:, kf, mc * P:(mc + 1) * P],
                        h1_sb[:, kf, :],
                        start=(kf == 0), stop=(kf == NFC - 1))
                f2 = tl_pool.tile([P, SH], F32, tag="f2")
                nc.scalar.activation(out=f2, in_=ps, func=AFT.Identity,
                                     bias=bf2_sb[:, mc:mc + 1])
                nc.vector.tensor_add(r_sb[:, mc, :], z_sb[:, mc, :], f2)

            layer_norm(r_sb, g2_sb, b2_sb, out_sb)

            nc.sync.dma_start(out=out.rearrange("(kc p) s -> p kc s", p=P),
                              in_=out_sb)


_NC_CACHE = None


def _get_nc():
    global _NC_CACHE
    if _NC_CACHE is None:
        _NC_CACHE = build_bass()
    return _NC_CACHE


def make_in_maps(lidar_features, image_features, Wq, bq, Wk, bk, Wv, bv,
                 Wo, bo, g1, b1, W1, bf1, W2, bf2, g2, b2):
    xT_full = np.ascontiguousarray(
        np.asarray(lidar_features, np.float32).reshape(C, S))
    y_full = np.ascontiguousarray(
        np.asarray(image_features, np.float32).reshape(C, S))
    common = {
        "y": y_full,
        "wq": np.ascontiguousarray(np.asarray(Wq, np.float32)),
        "wk": np.ascontiguousarray(np.asarray(Wk, np.float32)),
        "wv": np.ascontiguousarray(np.asarray(Wv, np.float32)),
        "wo": np.ascontiguousarray(np.asarray(Wo, np.float32)),
        "w1": np.ascontiguousarray(np.asarray(W1, np.float32)),
        "w2": np.ascontiguousarray(np.asarray(W2, np.float32)),
        "ones32": np.ones((P, 1), np.float32),
        "bq": np.asarray(bq, np.float32),
        "bv": np.asarray(bv, np.float32),
        "bo": np.asarray(bo, np.float32),
        "bf1": np.asarray(bf1, np.float32),
        "bf2": np.asarray(bf2, np.float32),
        "g1": np.asarray(g1, np.float32),
        "b1": np.asarray(b1, np.float32),
        "g2": np.asarray(g2, np.float32),
        "b2": np.asarray(b2, np.float32),
    }
    in_maps = []
    for c in range(NCORES):
        m = dict(common)
        m["xT"] = np.ascontiguousarray(xT_full[:, c * SH:(c + 1) * SH])
        in_maps.append(m)
    return in_maps


def kernel(lidar_features, image_features, Wq, bq, Wk, bk, Wv, bv, Wo, bo,
           g1, b1, W1, bf1, W2, bf2, g2, b2, num_heads, **run_kwargs):
    assert int(num_heads) == NH
    nc = _get_nc()
    in_maps = make_in_maps(lidar_features, image_features, Wq, bq, Wk, bk,
                           Wv, bv, Wo, bo, g1, b1, W1, bf1, W2, bf2, g2, b2)
    res = run_bass_kernel_spmd(nc, in_maps, core_ids=list(range(NCORES)),
                               **run_kwargs)
    full = np.concatenate([res.results[c]["out"] for c in range(NCORES)],
                          axis=1)
    kernel.last_results = res
    return full.reshape(1, C, 64, 64).astype(np.float32)


kernel.last_results = None


# revision 22
# speedup vs baseline: 4905.6593x; 4905.6593x over previous
"""Cross-attention fusion block on 8 trn2 NeuronCores.

Sharding: data-parallel over the query sequence (S=4096 -> 512 rows/core).
K/V projections are computed redundantly on every core (cheap vs attention).
Everything runs in channel-major ("transposed") layout [C, S] so that no
on-chip transposes are needed anywhere:
  inputs  lidar/image [1,C,H,W] -> [C, S]   (natural memory layout)
  output  [C, S] -> [1, C, H, W]            (natural memory layout)

Per-core pipeline (q = 512 query rows of this core), fp32r matmuls:
  qT = Wq^T @ xT (+bq)            [256, 512]
  kT = Wk^T @ y                   [256, 4096]   (bk dropped: softmax-invariant)
  v2 = y^T @ Wv, stored per-head as [V_h | 1]   (bv folded in after softmax)
  per 4-head group, per 128-row key chunk (ST tiles of 2 heads, row-packed
  4-way on the PE):
    ST[k,q]   = K_h chunk @ Q_h^T
    AT        = exp(ST / sqrt(32))              (ACT, no max subtraction)
    AVCS_h   += [V_h | 1]^T-chunk @ AT          (rows 0-31 attn@v, row 32
                                                 softmax denominator; one
                                                 accumulation chain per bank)
  attn_h = AV_h * (1/CS_h) + bv    (stage copy + DMA partition shifts; CS
                                    rows bounce through DRAM for broadcast)
  o = Wo^T @ attn (+bo); r = qT + o; z = LN(r)  (stats via ones-matmuls)
  h1 = relu(W1^T @ z + bf1); h2 = W2^T @ h1 + bf2; out = LN(z + h2)
"""

import sys

for _p in ("/opt/trn_rl_repo", "/opt/pypackages"):
    if _p not in sys.path:
        sys.path.append(_p)

import numpy as np

import concourse.bass as bass
import concourse.bacc as bacc
import concourse.tile as tile
from concourse import mybir
from concourse.bass_utils import run_bass_kernel_spmd

F32 = mybir.dt.float32
F32R = mybir.dt.float32r
AFT = mybir.ActivationFunctionType
ALU = mybir.AluOpType

P = 128           # SBUF partitions
C = 256           # channels
S = 4096          # sequence (64*64)
NCORES = 8
SH = S // NCORES  # 512 query rows per core
NH = 8            # heads
HD = 32           # head dim
HD1 = HD + 1      # V block plus the ones column for the colsum
F = 4 * C         # FFN hidden = 1024
NKC = C // P      # 2 channel chunks
NFC = F // P      # 8 ffn chunks
NSC = S // P      # 32 key chunks
EPS = 1e-5
INV_SQRT_HD = 1.0 / float(np.sqrt(HD))
INV_C = 1.0 / C


def build_bass():
    nc = bacc.Bacc()

    xT = nc.declare_dram_parameter("xT", [C, SH], F32R, isOutput=False)
    y = nc.declare_dram_parameter("y", [C, S], F32R, isOutput=False)
    w4 = nc.declare_dram_parameter("w4", [4, C, C], F32R, isOutput=False)
    w1 = nc.declare_dram_parameter("w1", [C, F], F32R, isOutput=False)
    w2 = nc.declare_dram_parameter("w2", [F, C], F32R, isOutput=False)
    ones32 = nc.declare_dram_parameter("ones32", [P, 1], F32R, isOutput=False)
    emat = nc.declare_dram_parameter("emat", [4, P], F32, isOutput=False)
    bpack = nc.declare_dram_parameter("bpack", [12, C], F32, isOutput=False)
    out = nc.declare_dram_parameter("out", [C, SH], F32, isOutput=True)

    with tile.TileContext(nc) as tc:
        _emit(tc, xT, y, w4, w1, w2, ones32, emat, bpack, out)
    if not nc.is_finalized():
        nc.finalize()
    return nc


def _emit(tc, xT, y, w4, w1, w2, ones32, emat, bpack, out):
    nc = tc.nc

    import contextlib
    stack = contextlib.ExitStack()
    with stack:
        consts = stack.enter_context(tc.tile_pool(name="consts", bufs=1))
        big = stack.enter_context(tc.tile_pool(name="big", bufs=1))

        # ---------------- constants / weights into SBUF ----------------
        y_sb = big.tile([P, NKC, S], F32R)         # y[ch, s]; ch = kc*128 + p
        y_r = y.rearrange("(kc p) s -> p kc s", p=P)
        HS = S // 2
        for kc in range(NKC):
            for sh2 in range(2):
                eng = [nc.sync, nc.gpsimd, nc.sync, nc.gpsimd][kc * 2 + sh2]
                eng.dma_start(
                    out=y_sb[:, kc, sh2 * HS:(sh2 + 1) * HS],
                    in_=y_r[:, kc, sh2 * HS:(sh2 + 1) * HS])
        xT_sb = big.tile([P, NKC, SH], F32R)
        nc.sync.dma_start(out=xT_sb, in_=xT.rearrange("(kc p) s -> p kc s", p=P))

        w4_sb = consts.tile([P, 4, NKC, C], F32R)
        nc.gpsimd.dma_start(
            out=w4_sb, in_=w4.rearrange("w (kc p) m -> p w kc m", p=P))
        wq_sb, wk_sb, wv_sb, wo_sb = (w4_sb[:, i] for i in range(4))
        w1_sb = consts.tile([P, NKC, F], F32R)
        nc.gpsimd.dma_start(out=w1_sb, in_=w1.rearrange("(kc p) m -> p kc m", p=P))
        w2_sb = consts.tile([P, NFC, C], F32R)
        nc.gpsimd.dma_start(out=w2_sb, in_=w2.rearrange("(kc p) m -> p kc m", p=P))

        bp_sb = consts.tile([P, 12, NKC], F32)
        nc.gpsimd.dma_start(
            out=bp_sb, in_=bpack.rearrange("n (kc p) -> p n kc", p=P))
        bq_sb, bv_sb, bo_sb, bf2_sb = (bp_sb[:, i] for i in range(4))
        g1_sb, b1_sb, g2_sb, b2_sb = (bp_sb[:, i] for i in range(4, 8))

        ones1r = consts.tile([P, 1], F32R)      # LN-stats lhsT (f32r ones)
        nc.gpsimd.dma_start(out=ones1r, in_=ones32[:])
        emat_sb = consts.tile([4, P], F32)      # head-broadcast matrix
        nc.gpsimd.dma_start(out=emat_sb, in_=emat[:])
        ones_rep = consts.tile([1, P], F32)     # K=1 row-replication lhsT
        nc.vector.memset(ones_rep, 1.0)
        eps_sb = consts.tile([P, 1], F32)
        nc.vector.memset(eps_sb, EPS)

        # persistent activations
        qT_sb = big.tile([P, NKC, SH], F32R)    # q^T  (with bq)
        kT_sb = big.tile([P, NKC, S], F32R)     # k^T  (no bk; softmax-invariant)
        v2_sb = big.tile([P, NSC, NH, HD1], F32R)  # per head [V_h | 1]
        attn_sb = big.tile([P, NKC, SH], F32R)  # (attn @ v)^T + bv
        z_sb = big.tile([P, NKC, SH], F32R)     # LN1 output
        h1_sb = big.tile([P, NFC, SH], F32R)    # relu(ffn1)
        out_sb = big.tile([P, NKC, SH], F32)    # final
        r_sb = big.tile([P, NKC, SH], F32R)     # residual sums (LN inputs)

        # ones column of v2 (memset cannot write f32r -> DMA broadcast)
        ones_ap = ones32[:]
        ones_col = bass.AP(
            tensor=ones_ap.tensor, offset=ones_ap.offset,
            ap=[ones_ap.ap[0], [0, NSC * NH]])
        nc.sync.dma_start(out=v2_sb[:, :, :, HD:HD1].squeeze(),
                          in_=ones_col)

        # ---------------- preamble: qT, kT, v projections ----------------
        with tc.tile_pool(name="pre_k", bufs=3, space="PSUM") as pre_k, \
             tc.tile_pool(name="pre_v", bufs=3, space="PSUM") as pre_v:
            # q^T[c',q] = sum_ch Wq[ch,c'] xT[ch,q]
            for mc in range(NKC):
                ps = pre_k.tile([P, SH], F32, tag="ps_k")
                for kc in range(NKC):
                    nc.tensor.matmul(
                        ps, wq_sb[:, kc, mc * P:(mc + 1) * P],
                        xT_sb[:, kc, :],
                        start=(kc == 0), stop=(kc == NKC - 1))
                nc.scalar.activation(out=qT_sb[:, mc, :], in_=ps,
                                     func=AFT.Identity,
                                     bias=bq_sb[:, mc:mc + 1])
            # k^T[c',s] = sum_ch Wk[ch,c'] y[ch,s]   in 512-col blocks
            for sb in range(S // 512):
                for mc in range(NKC):
                    ps = pre_k.tile([P, 512], F32, tag="ps_k")
                    for kc in range(NKC):
                        nc.tensor.matmul(
                            ps, wk_sb[:, kc, mc * P:(mc + 1) * P],
                            y_sb[:, kc, sb * 512:(sb + 1) * 512],
                            start=(kc == 0), stop=(kc == NKC - 1))
                    nc.vector.tensor_copy(
                        kT_sb[:, mc, sb * 512:(sb + 1) * 512], ps)
            # v[s,c'] = sum_ch y[ch,s] Wv[ch,c']    per 128-row s chunk
            for ck in range(NSC):
                ps = pre_v.tile([P, C], F32, tag="ps_v")
                for kc in range(NKC):
                    nc.tensor.matmul(
                        ps, y_sb[:, kc, ck * P:(ck + 1) * P],
                        wv_sb[:, kc, :],
                        start=(kc == 0), stop=(kc == NKC - 1))
                # scatter the 8 per-head blocks into the [V_h | 1] layout
                dst = v2_sb[:, ck, :, 0:HD]
                nc.vector.tensor_copy(dst, ps.rearrange("p (h d) -> p h d",
                                                        d=HD))

        # ---------------- attention ----------------
        with tc.tile_pool(name="st", bufs=2, space="PSUM") as st_pool, \
             tc.tile_pool(name="avcs", bufs=4, space="PSUM") as avcs_pool, \
             tc.tile_pool(name="at", bufs=4) as at_pool, \
             tc.tile_pool(name="nrm", bufs=1) as nrm_pool:
            for grp in range(2):
                avcs = [avcs_pool.tile([HD1, SH], F32, tag="avcs",
                                       name=f"avcs_g{grp}_{j}")
                        for j in range(4)]
                for ck in range(NSC):
                    for pair in range(2):
                        st = st_pool.tile([P, 2, SH], F32, tag="st")
                        for j in range(2):
                            h = 4 * grp + 2 * pair + j
                            po = HD * (h % 4)
                            nc.tensor.matmul(
                                st[:, j, :],
                                kT_sb[po:po + HD, grp, ck * P:(ck + 1) * P],
                                qT_sb[po:po + HD, grp, :],
                                start=True, stop=True,
                                tile_position=(po, 0))
                        at = at_pool.tile([P, 2, SH], F32R, tag="at")
                        nc.scalar.activation(out=at, in_=st, func=AFT.Exp,
                                             scale=INV_SQRT_HD)
                        for j in range(2):
                            h = 4 * grp + 2 * pair + j
                            nc.tensor.matmul(
                                avcs[2 * pair + j],
                                v2_sb[:, ck, h, :],
                                at[:, j, :],
                                start=(ck == 0), stop=(ck == NSC - 1))
                # normalize: attn_h = av_h / cs_h (+ bv later, whole group)
                av_all = nrm_pool.tile([P, SH], F32, tag="av_all")
                stage = nrm_pool.tile([HD1, 4, SH], F32, tag="stage")
                for j in range(4):
                    nc.vector.tensor_copy(stage[:, j, :], avcs[j])
                    eng = [nc.sync, nc.gpsimd, nc.sync, nc.gpsimd][j]
                    eng.dma_start(out=av_all[HD * j:HD * (j + 1), :],
                                  in_=stage[0:HD, j, :])
                cs4 = nrm_pool.tile([4, SH], F32, tag="cs4")
                nc.sync.dma_start(out=cs4, in_=stage[HD:HD1, :, :])
                rec4 = nrm_pool.tile([4, SH], F32, tag="rec4")
                scr4 = nrm_pool.tile([4, SH], F32, tag="scr4")
                nc.vector.reciprocal_approx_accurate(out=rec4, in_=cs4,
                                                     scratch=scr4)
                rec_all = st_pool.tile([P, SH], F32, tag="st",
                                       name=f"rec_all_{grp}")
                nc.tensor.matmul(rec_all, emat_sb, rec4, start=True, stop=True)
                tmp = nrm_pool.tile([P, SH], F32, tag="tmp")
                nc.vector.tensor_mul(tmp, av_all, rec_all)
                nc.vector.tensor_scalar_add(out=attn_sb[:, grp, :], in0=tmp,
                                            scalar1=bv_sb[:, grp:grp + 1])

        # ---------------- tail: out-proj, LN1, FFN, LN2 ----------------
        with tc.tile_pool(name="mm", bufs=3, space="PSUM") as mm_pool, \
             tc.tile_pool(name="stat", bufs=1, space="PSUM") as stat_pool, \
             tc.tile_pool(name="rep", bufs=1, space="PSUM") as rep_pool, \
             tc.tile_pool(name="tl", bufs=2) as tl_pool, \
             tc.tile_pool(name="tr", bufs=1) as tr_pool:

            def layer_norm(x3, gamma, beta, out3):
                """out3 = LN(x3) over the channel axis (2 chunks of 128)."""
                mu_ps = stat_pool.tile([1, SH], F32, tag="mu")
                e2_ps = stat_pool.tile([1, SH], F32, tag="e2")
                for kc in range(NKC):
                    nc.tensor.matmul(mu_ps, ones1r, x3[:, kc, :],
                                     start=(kc == 0), stop=(kc == NKC - 1))
                for kc in range(NKC):
                    sq = tl_pool.tile([P, SH], F32R, tag="sq")
                    nc.vector.tensor_mul(sq, x3[:, kc, :], x3[:, kc, :])
                    nc.tensor.matmul(e2_ps, ones1r, sq,
                                     start=(kc == 0), stop=(kc == NKC - 1))
                mu_row = tr_pool.tile([1, SH], F32, tag="mu_row")
                nc.vector.tensor_scalar_mul(out=mu_row, in0=mu_ps,
                                            scalar1=INV_C)
                mu2_row = tr_pool.tile([1, SH], F32, tag="mu2_row")
                nc.vector.tensor_mul(mu2_row, mu_row, mu_row)
                var_row = tr_pool.tile([1, SH], F32, tag="var_row")
                # var = E[x^2] - mu^2 = e2/C - mu^2
                nc.vector.scalar_tensor_tensor(
                    out=var_row, in0=e2_ps, scalar=INV_C, in1=mu2_row,
                    op0=ALU.mult, op1=ALU.subtract)
                std_row = tr_pool.tile([1, SH], F32, tag="std_row")
                nc.scalar.activation(out=std_row, in_=var_row, func=AFT.Sqrt,
                                     bias=eps_sb[:1, :])
                rstd_row = tr_pool.tile([1, SH], F32, tag="rstd_row")
                scr_row = tr_pool.tile([1, SH], F32, tag="mu2_row")
                nc.vector.reciprocal_approx_accurate(out=rstd_row, in_=std_row,
                                                     scratch=scr_row)
                mu_rep = rep_pool.tile([P, SH], F32, tag="mu_rep")
                nc.tensor.matmul(mu_rep, ones_rep, mu_row,
                                 start=True, stop=True)
                rstd_rep = rep_pool.tile([P, SH], F32, tag="rstd_rep")
                nc.tensor.matmul(rstd_rep, ones_rep, rstd_row,
                                 start=True, stop=True)
                for kc in range(NKC):
                    t = tl_pool.tile([P, SH], F32, tag="t")
                    nc.vector.tensor_sub(t, x3[:, kc, :], mu_rep)
                    t2 = tl_pool.tile([P, SH], F32, tag="t2")
                    nc.vector.tensor_mul(t2, t, rstd_rep)
                    nc.vector.tensor_scalar(
                        out=out3[:, kc, :], in0=t2,
                        scalar1=gamma[:, kc:kc + 1], scalar2=beta[:, kc:kc + 1],
                        op0=ALU.mult, op1=ALU.add)

            # out-projection + residual (r = qT + Wo^T attn + bo)
            for mc in range(NKC):
                ps = mm_pool.tile([P, SH], F32, tag="mm")
                for kc in range(NKC):
                    nc.tensor.matmul(
                        ps, wo_sb[:, kc, mc * P:(mc + 1) * P],
                        attn_sb[:, kc, :],
                        start=(kc == 0), stop=(kc == NKC - 1))
                o_t = tl_pool.tile([P, SH], F32, tag="o_t")
                nc.vector.tensor_scalar_add(out=o_t, in0=ps,
                                            scalar1=bo_sb[:, mc:mc + 1])
                nc.vector.tensor_add(r_sb[:, mc, :], qT_sb[:, mc, :], o_t)

            layer_norm(r_sb, g1_sb, b1_sb, z_sb)

            # FFN1 + relu
            for mf in range(NFC):
                ps = mm_pool.tile([P, SH], F32, tag="mm")
                for kc in range(NKC):
                    nc.tensor.matmul(
                        ps, w1_sb[:, kc, mf * P:(mf + 1) * P],
                        z_sb[:, kc, :],
                        start=(kc == 0), stop=(kc == NKC - 1))
                nc.vector.tensor_scalar(
                    out=h1_sb[:, mf, :], in0=ps,
                    scalar1=bp_sb[:, 8 + mf // 2, mf % 2:mf % 2 + 1],
                    scalar2=0.0,
                    op0=ALU.add, op1=ALU.max)
            # FFN2 + bias + residual
            for mc in range(NKC):
                ps = mm_pool.tile([P, SH], F32, tag="mm")
                for kf in range(NFC):
                    nc.tensor.matmul(
                        ps, w2_sb[:, kf, mc * P:(mc + 1) * P],
                        h1_sb[:, kf, :],
                        start=(kf == 0), stop=(kf == NFC - 1))
                f2 = tl_pool.tile([P, SH], F32, tag="f2")
                nc.vector.tensor_scalar_add(out=f2, in0=ps,
                                            scalar1=bf2_sb[:, mc:mc + 1])
                nc.vector.tensor_add(r_sb[:, mc, :], z_sb[:, mc, :], f2)

            layer_norm(r_sb, g2_sb, b2_sb, out_sb)

            nc.sync.dma_start(out=out.rearrange("(kc p) s -> p kc s", p=P),
                              in_=out_sb)


_NC_CACHE = None


def _get_nc():
    global _NC_CACHE
    if _NC_CACHE is None:
        _NC_CACHE = build_bass()
    return _NC_CACHE


def make_in_maps(lidar_features, image_features, Wq, bq, Wk, bk, Wv, bv,
                 Wo, bo, g1, b1, W1, bf1, W2, bf2, g2, b2):
    xT_full = np.ascontiguousarray(
        np.asarray(lidar_features, np.float32).reshape(C, S))
    y_full = np.ascontiguousarray(
        np.asarray(image_features, np.float32).reshape(C, S))
    w4 = np.ascontiguousarray(np.stack([
        np.asarray(Wq, np.float32), np.asarray(Wk, np.float32),
        np.asarray(Wv, np.float32), np.asarray(Wo, np.float32)]))
    bpack = np.ascontiguousarray(np.concatenate([
        np.asarray(bq, np.float32)[None], np.asarray(bv, np.float32)[None],
        np.asarray(bo, np.float32)[None], np.asarray(bf2, np.float32)[None],
        np.asarray(g1, np.float32)[None], np.asarray(b1, np.float32)[None],
        np.asarray(g2, np.float32)[None], np.asarray(b2, np.float32)[None],
        np.asarray(bf1, np.float32).reshape(4, C)]))
    em = np.zeros((4, P), np.float32)
    for j in range(4):
        em[j, HD * j:HD * (j + 1)] = 1.0
    common = {
        "y": y_full,
        "emat": em,
        "w4": w4,
        "w1": np.ascontiguousarray(np.asarray(W1, np.float32)),
        "w2": np.ascontiguousarray(np.asarray(W2, np.float32)),
        "ones32": np.ones((P, 1), np.float32),
        "bpack": bpack,
    }
    in_maps = []
    for c in range(NCORES):
        m = dict(common)
        m["xT"] = np.ascontiguousarray(xT_full[:, c * SH:(c + 1) * SH])
        in_maps.append(m)
    return in_maps


def kernel(lidar_features, image_features, Wq, bq, Wk, bk, Wv, bv, Wo, bo,
           g1, b1, W1, bf1, W2, bf2, g2, b2, num_heads, **run_kwargs):
    assert int(num_heads) == NH
    nc = _get_nc()
    in_maps = make_in_maps(lidar_features, image_features, Wq, bq, Wk, bk,
                           Wv, bv, Wo, bo, g1, b1, W1, bf1, W2, bf2, g2, b2)
    res = run_bass_kernel_spmd(nc, in_maps, core_ids=list(range(NCORES)),
                               **run_kwargs)
    full = np.concatenate([res.results[c]["out"] for c in range(NCORES)],
                          axis=1)
    kernel.last_results = res
    return full.reshape(1, C, 64, 64).astype(np.float32)


kernel.last_results = None


# revision 23
# speedup vs baseline: 5159.7594x; 1.0518x over previous
"""Cross-attention fusion block on 8 trn2 NeuronCores.

Sharding: data-parallel over the query sequence (S=4096 -> 512 rows/core).
K/V projections are computed redundantly on every core (cheap vs attention).
Everything runs in channel-major ("transposed") layout [C, S] so that no
on-chip transposes are needed anywhere:
  inputs  lidar/image [1,C,H,W] -> [C, S]   (natural memory layout)
  output  [C, S] -> [1, C, H, W]            (natural memory layout)

Per-core pipeline (q = 512 query rows of this core), fp32r matmuls:
  qT = Wq^T @ xT (+bq)            [256, 512]
  kT = Wk^T @ y                   [256, 4096]   (bk dropped: softmax-invariant)
  v2 = y^T @ Wv, stored per-head as [V_h | 1]   (bv folded in after softmax)
  per 4-head group, per 128-row key chunk (ST tiles of 2 heads, row-packed
  4-way on the PE):
    ST[k,q]   = K_h chunk @ Q_h^T
    AT        = exp(ST / sqrt(32))              (ACT, no max subtraction)
    AVCS_h   += [V_h | 1]^T-chunk @ AT          (rows 0-31 attn@v, row 32
                                                 softmax denominator; one
                                                 accumulation chain per bank)
  attn_h = AV_h * (1/CS_h) + bv    (stage copy + DMA partition shifts; CS
                                    rows bounce through DRAM for broadcast)
  o = Wo^T @ attn (+bo); r = qT + o; z = LN(r)  (stats via ones-matmuls)
  h1 = relu(W1^T @ z + bf1); h2 = W2^T @ h1 + bf2; out = LN(z + h2)
"""

import sys

for _p in ("/opt/trn_rl_repo", "/opt/pypackages"):
    if _p not in sys.path:
        sys.path.append(_p)

import numpy as np

import concourse.bass as bass
import concourse.bacc as bacc
import concourse.tile as tile
from concourse import mybir
from concourse.bass_utils import run_bass_kernel_spmd

F32 = mybir.dt.float32
F32R = mybir.dt.float32r
AFT = mybir.ActivationFunctionType
ALU = mybir.AluOpType

P = 128           # SBUF partitions
C = 256           # channels
S = 4096          # sequence (64*64)
NCORES = 8
SH = S // NCORES  # 512 query rows per core
NH = 8            # heads
HD = 32           # head dim
HD1 = HD + 1      # V block plus the ones column for the colsum
F = 4 * C         # FFN hidden = 1024
NKC = C // P      # 2 channel chunks
NFC = F // P      # 8 ffn chunks
NSC = S // P      # 32 key chunks
EPS = 1e-5
INV_SQRT_HD = 1.0 / float(np.sqrt(HD))
INV_C = 1.0 / C


def build_bass():
    nc = bacc.Bacc()

    xT = nc.declare_dram_parameter("xT", [C, SH], F32R, isOutput=False)
    y = nc.declare_dram_parameter("y", [C, S], F32R, isOutput=False)
    w4 = nc.declare_dram_parameter("w4", [4, C, C], F32R, isOutput=False)
    w1 = nc.declare_dram_parameter("w1", [C, F], F32R, isOutput=False)
    w2 = nc.declare_dram_parameter("w2", [F, C], F32R, isOutput=False)
    ones32 = nc.declare_dram_parameter("ones32", [P, 1], F32R, isOutput=False)
    emat = nc.declare_dram_parameter("emat", [4, P], F32, isOutput=False)
    bpack = nc.declare_dram_parameter("bpack", [12, C], F32, isOutput=False)
    out = nc.declare_dram_parameter("out", [C, SH], F32, isOutput=True)

    with tile.TileContext(nc) as tc:
        _emit(tc, xT, y, w4, w1, w2, ones32, emat, bpack, out)
    if not nc.is_finalized():
        nc.finalize()
    return nc


def _emit(tc, xT, y, w4, w1, w2, ones32, emat, bpack, out):
    nc = tc.nc

    import contextlib
    stack = contextlib.ExitStack()
    with stack:
        consts = stack.enter_context(tc.tile_pool(name="consts", bufs=1))
        big = stack.enter_context(tc.tile_pool(name="big", bufs=1))

        # ---------------- constants / weights into SBUF ----------------
        y_sb = big.tile([P, NKC, S], F32R)         # y[ch, s]; ch = kc*128 + p
        y_r = y.rearrange("(kc p) s -> p kc s", p=P)
        HS = S // 2
        for kc in range(NKC):
            for sh2 in range(2):
                eng = [nc.sync, nc.gpsimd, nc.sync, nc.gpsimd][kc * 2 + sh2]
                eng.dma_start(
                    out=y_sb[:, kc, sh2 * HS:(sh2 + 1) * HS],
                    in_=y_r[:, kc, sh2 * HS:(sh2 + 1) * HS])
        xT_sb = big.tile([P, NKC, SH], F32R)
        nc.sync.dma_start(out=xT_sb, in_=xT.rearrange("(kc p) s -> p kc s", p=P))

        w4_sb = consts.tile([P, 4, NKC, C], F32R)
        nc.gpsimd.dma_start(
            out=w4_sb, in_=w4.rearrange("w (kc p) m -> p w kc m", p=P))
        wq_sb, wk_sb, wv_sb, wo_sb = (w4_sb[:, i] for i in range(4))
        w1_sb = consts.tile([P, NKC, F], F32R)
        nc.gpsimd.dma_start(out=w1_sb, in_=w1.rearrange("(kc p) m -> p kc m", p=P))
        w2_sb = consts.tile([P, NFC, C], F32R)
        nc.gpsimd.dma_start(out=w2_sb, in_=w2.rearrange("(kc p) m -> p kc m", p=P))

        bp_sb = consts.tile([P, 12, NKC], F32)
        nc.gpsimd.dma_start(
            out=bp_sb, in_=bpack.rearrange("n (kc p) -> p n kc", p=P))
        bq_sb, bv_sb, bo_sb, bf2_sb = (bp_sb[:, i] for i in range(4))
        g1_sb, b1_sb, g2_sb, b2_sb = (bp_sb[:, i] for i in range(4, 8))

        ones1r = consts.tile([P, 1], F32R)      # LN-stats lhsT (f32r ones)
        nc.gpsimd.dma_start(out=ones1r, in_=ones32[:])
        emat_sb = consts.tile([4, P], F32)      # head-broadcast matrix
        nc.gpsimd.dma_start(out=emat_sb, in_=emat[:])
        ones_rep = consts.tile([1, P], F32)     # K=1 row-replication lhsT
        nc.vector.memset(ones_rep, 1.0)
        eps_sb = consts.tile([P, 1], F32)
        nc.vector.memset(eps_sb, EPS)

        # persistent activations
        qT_sb = big.tile([P, NKC, SH], F32R)    # q^T  (with bq)
        kT_sb = big.tile([P, NKC, S], F32R)     # k^T  (no bk; softmax-invariant)
        v2_sb = big.tile([P, NSC, NH, HD1], F32R)  # per head [V_h | 1]
        attn_sb = big.tile([P, NKC, SH], F32R)  # (attn @ v)^T + bv
        z_sb = big.tile([P, NKC, SH], F32R)     # LN1 output
        h1_sb = big.tile([P, NFC, SH], F32R)    # relu(ffn1)
        out_sb = big.tile([P, NKC, SH], F32)    # final
        r_sb = big.tile([P, NKC, SH], F32R)     # residual sums (LN inputs)

        # ones column of v2 (memset cannot write f32r -> DMA broadcast)
        ones_ap = ones32[:]
        ones_col = bass.AP(
            tensor=ones_ap.tensor, offset=ones_ap.offset,
            ap=[ones_ap.ap[0], [0, NSC * NH]])
        nc.sync.dma_start(out=v2_sb[:, :, :, HD:HD1].squeeze(),
                          in_=ones_col)

        # ---------------- preamble: qT, kT, v projections ----------------
        with tc.tile_pool(name="pre_k", bufs=3, space="PSUM") as pre_k, \
             tc.tile_pool(name="pre_v", bufs=3, space="PSUM") as pre_v:
            # q^T[c',q] = sum_ch Wq[ch,c'] xT[ch,q]
            for mc in range(NKC):
                ps = pre_k.tile([P, SH], F32, tag="ps_k")
                for kc in range(NKC):
                    nc.tensor.matmul(
                        ps, wq_sb[:, kc, mc * P:(mc + 1) * P],
                        xT_sb[:, kc, :],
                        start=(kc == 0), stop=(kc == NKC - 1))
                nc.scalar.activation(out=qT_sb[:, mc, :], in_=ps,
                                     func=AFT.Identity,
                                     bias=bq_sb[:, mc:mc + 1])
            # k^T[c',s] = sum_ch Wk[ch,c'] y[ch,s]   in 512-col blocks
            for sb in range(S // 512):
                for mc in range(NKC):
                    ps = pre_k.tile([P, 512], F32, tag="ps_k")
                    for kc in range(NKC):
                        nc.tensor.matmul(
                            ps, wk_sb[:, kc, mc * P:(mc + 1) * P],
                            y_sb[:, kc, sb * 512:(sb + 1) * 512],
                            start=(kc == 0), stop=(kc == NKC - 1))
                    nc.vector.tensor_copy(
                        kT_sb[:, mc, sb * 512:(sb + 1) * 512], ps)
            # v[s,c'] = sum_ch y[ch,s] Wv[ch,c']    per 128-row s chunk
            for ck in range(NSC):
                ps = pre_v.tile([P, C], F32, tag="ps_v")
                for kc in range(NKC):
                    nc.tensor.matmul(
                        ps, y_sb[:, kc, ck * P:(ck + 1) * P],
                        wv_sb[:, kc, :],
                        start=(kc == 0), stop=(kc == NKC - 1))
                # scatter the 8 per-head blocks into the [V_h | 1] layout
                dst = v2_sb[:, ck, :, 0:HD]
                nc.vector.tensor_copy(dst, ps.rearrange("p (h d) -> p h d",
                                                        d=HD))

        # ---------------- attention ----------------
        with tc.tile_pool(name="st", bufs=2, space="PSUM") as st_pool, \
             tc.tile_pool(name="avcs", bufs=4, space="PSUM") as avcs_pool, \
             tc.tile_pool(name="at", bufs=4) as at_pool, \
             tc.tile_pool(name="nrm", bufs=1) as nrm_pool:
            for grp in range(2):
                avcs = [avcs_pool.tile([HD1, SH], F32, tag="avcs",
                                       name=f"avcs_g{grp}_{j}")
                        for j in range(4)]
                for ck in range(NSC):
                    for pair in range(2):
                        st = st_pool.tile([P, 2, SH], F32, tag="st")
                        for j in range(2):
                            h = 4 * grp + 2 * pair + j
                            po = HD * (h % 4)
                            nc.tensor.matmul(
                                st[:, j, :],
                                kT_sb[po:po + HD, grp, ck * P:(ck + 1) * P],
                                qT_sb[po:po + HD, grp, :],
                                start=True, stop=True,
                                tile_position=(po, 0))
                        at = at_pool.tile([P, 2, SH], F32R, tag="at")
                        nc.scalar.activation(out=at, in_=st, func=AFT.Exp,
                                             scale=INV_SQRT_HD)
                        for j in range(2):
                            h = 4 * grp + 2 * pair + j
                            nc.tensor.matmul(
                                avcs[2 * pair + j],
                                v2_sb[:, ck, h, :],
                                at[:, j, :],
                                start=(ck == 0), stop=(ck == NSC - 1))
                # normalize: attn_h = av_h / cs_h (+ bv later, whole group)
                av_all = nrm_pool.tile([P, SH], F32, tag="av_all")
                stage = nrm_pool.tile([HD1, 4, SH], F32, tag="stage")
                for j in range(4):
                    nc.vector.tensor_copy(stage[:, j, :], avcs[j])
                    eng = [nc.sync, nc.gpsimd, nc.sync, nc.gpsimd][j]
                    eng.dma_start(out=av_all[HD * j:HD * (j + 1), :],
                                  in_=stage[0:HD, j, :])
                cs4 = nrm_pool.tile([4, SH], F32, tag="cs4")
                nc.sync.dma_start(out=cs4, in_=stage[HD:HD1, :, :])
                rec4 = nrm_pool.tile([4, SH], F32, tag="rec4")
                scr4 = nrm_pool.tile([4, SH], F32, tag="scr4")
                nc.vector.reciprocal_approx_accurate(out=rec4, in_=cs4,
                                                     scratch=scr4)
                rec_all = avcs_pool.tile([P, SH], F32, tag="avcs",
                                         name=f"rec_all_{grp}")
                nc.tensor.matmul(rec_all, emat_sb, rec4, start=True, stop=True)
                tmp = nrm_pool.tile([P, SH], F32, tag="tmp")
                nc.vector.tensor_mul(tmp, av_all, rec_all)
                nc.vector.tensor_scalar_add(out=attn_sb[:, grp, :], in0=tmp,
                                            scalar1=bv_sb[:, grp:grp + 1])

        # ---------------- tail: out-proj, LN1, FFN, LN2 ----------------
        with tc.tile_pool(name="mm", bufs=3, space="PSUM") as mm_pool, \
             tc.tile_pool(name="stat", bufs=1, space="PSUM") as stat_pool, \
             tc.tile_pool(name="rep", bufs=1, space="PSUM") as rep_pool, \
             tc.tile_pool(name="tl", bufs=2) as tl_pool, \
             tc.tile_pool(name="tr", bufs=1) as tr_pool:

            def layer_norm(x3, gamma, beta, out3):
                """out3 = LN(x3) over the channel axis (2 chunks of 128)."""
                mu_ps = stat_pool.tile([1, SH], F32, tag="mu")
                e2_ps = stat_pool.tile([1, SH], F32, tag="e2")
                for kc in range(NKC):
                    nc.tensor.matmul(mu_ps, ones1r, x3[:, kc, :],
                                     start=(kc == 0), stop=(kc == NKC - 1))
                for kc in range(NKC):
                    sq = tl_pool.tile([P, SH], F32R, tag="sq")
                    if kc == 0:
                        nc.scalar.activation(out=sq, in_=x3[:, kc, :],
                                             func=AFT.Square)
                    else:
                        nc.vector.tensor_mul(sq, x3[:, kc, :], x3[:, kc, :])
                    nc.tensor.matmul(e2_ps, ones1r, sq,
                                     start=(kc == 0), stop=(kc == NKC - 1))
                mu_row = tr_pool.tile([1, SH], F32, tag="mu_row")
                nc.vector.tensor_scalar_mul(out=mu_row, in0=mu_ps,
                                            scalar1=INV_C)
                mu2_row = tr_pool.tile([1, SH], F32, tag="mu2_row")
                nc.vector.tensor_mul(mu2_row, mu_row, mu_row)
                var_row = tr_pool.tile([1, SH], F32, tag="var_row")
                # var = E[x^2] - mu^2 = e2/C - mu^2
                nc.vector.scalar_tensor_tensor(
                    out=var_row, in0=e2_ps, scalar=INV_C, in1=mu2_row,
                    op0=ALU.mult, op1=ALU.subtract)
                std_row = tr_pool.tile([1, SH], F32, tag="std_row")
                nc.scalar.activation(out=std_row, in_=var_row, func=AFT.Sqrt,
                                     bias=eps_sb[:1, :])
                rstd_row = tr_pool.tile([1, SH], F32, tag="rstd_row")
                scr_row = tr_pool.tile([1, SH], F32, tag="mu2_row")
                nc.vector.reciprocal_approx_accurate(out=rstd_row, in_=std_row,
                                                     scratch=scr_row)
                mu_rep = rep_pool.tile([P, SH], F32, tag="mu_rep")
                nc.tensor.matmul(mu_rep, ones_rep, mu_row,
                                 start=True, stop=True)
                rstd_rep = rep_pool.tile([P, SH], F32, tag="rstd_rep")
                nc.tensor.matmul(rstd_rep, ones_rep, rstd_row,
                                 start=True, stop=True)
                for kc in range(NKC):
                    t = tl_pool.tile([P, SH], F32, tag="t")
                    nc.vector.tensor_sub(t, x3[:, kc, :], mu_rep)
                    t2 = tl_pool.tile([P, SH], F32, tag="t2")
                    nc.vector.tensor_mul(t2, t, rstd_rep)
                    nc.vector.tensor_scalar(
                        out=out3[:, kc, :], in0=t2,
                        scalar1=gamma[:, kc:kc + 1], scalar2=beta[:, kc:kc + 1],
                        op0=ALU.mult, op1=ALU.add)

            # out-projection + residual (r = qT + Wo^T attn + bo)
            for mc in range(NKC):
                ps = mm_pool.tile([P, SH], F32, tag="mm")
                for kc in range(NKC):
                    nc.tensor.matmul(
                        ps, wo_sb[:, kc, mc * P:(mc + 1) * P],
                        attn_sb[:, kc, :],
                        start=(kc == 0), stop=(kc == NKC - 1))
                o_t = tl_pool.tile([P, SH], F32, tag="o_t")
                nc.vector.tensor_scalar_add(out=o_t, in0=ps,
                                            scalar1=bo_sb[:, mc:mc + 1])
                nc.vector.tensor_add(r_sb[:, mc, :], qT_sb[:, mc, :], o_t)

            layer_norm(r_sb, g1_sb, b1_sb, z_sb)

            # FFN1 + relu
            for mf in range(NFC):
                ps = mm_pool.tile([P, SH], F32, tag="mm")
                for kc in range(NKC):
                    nc.tensor.matmul(
                        ps, w1_sb[:, kc, mf * P:(mf + 1) * P],
                        z_sb[:, kc, :],
                        start=(kc == 0), stop=(kc == NKC - 1))
                if mf % 2 == 0:
                    nc.scalar.activation(
                        out=h1_sb[:, mf, :], in_=ps, func=AFT.Relu,
                        bias=bp_sb[:, 8 + mf // 2, mf % 2:mf % 2 + 1])
                else:
                    nc.vector.tensor_scalar(
                        out=h1_sb[:, mf, :], in0=ps,
                        scalar1=bp_sb[:, 8 + mf // 2, mf % 2:mf % 2 + 1],
                        scalar2=0.0,
                        op0=ALU.add, op1=ALU.max)
            # FFN2 + bias + residual
            for mc in range(NKC):
                ps = mm_pool.tile([P, SH], F32, tag="mm")
                for kf in range(NFC):
                    nc.tensor.matmul(
                        ps, w2_sb[:, kf, mc * P:(mc + 1) * P],
                        h1_sb[:, kf, :],
                        start=(kf == 0), stop=(kf == NFC - 1))
                f2 = tl_pool.tile([P, SH], F32, tag="f2")
                nc.vector.tensor_scalar_add(out=f2, in0=ps,
                                            scalar1=bf2_sb[:, mc:mc + 1])
                nc.vector.tensor_add(r_sb[:, mc, :], z_sb[:, mc, :], f2)

            layer_norm(r_sb, g2_sb, b2_sb, out_sb)

            out_r = out.rearrange("(kc p) s -> p kc s", p=P)
            nc.sync.dma_start(out=out_r[:, 0, :], in_=out_sb[:, 0, :])
            nc.gpsimd.dma_start(out=out_r[:, 1, :], in_=out_sb[:, 1, :])


_NC_CACHE = None


def _get_nc():
    global _NC_CACHE
    if _NC_CACHE is None:
        _NC_CACHE = build_bass()
    return _NC_CACHE


def make_in_maps(lidar_features, image_features, Wq, bq, Wk, bk, Wv, bv,
                 Wo, bo, g1, b1, W1, bf1, W2, bf2, g2, b2):
    xT_full = np.ascontiguousarray(
        np.asarray(lidar_features, np.float32).reshape(C, S))
    y_full = np.ascontiguousarray(
        np.asarray(image_features, np.float32).reshape(C, S))
    w4 = np.ascontiguousarray(np.stack([
        np.asarray(Wq, np.float32), np.asarray(Wk, np.float32),
        np.asarray(Wv, np.float32), np.asarray(Wo, np.float32)]))
    bpack = np.ascontiguousarray(np.concatenate([
        np.asarray(bq, np.float32)[None], np.asarray(bv, np.float32)[None],
        np.asarray(bo, np.float32)[None], np.asarray(bf2, np.float32)[None],
        np.asarray(g1, np.float32)[None], np.asarray(b1, np.float32)[None],
        np.asarray(g2, np.float32)[None], np.asarray(b2, np.float32)[None],
        np.asarray(bf1, np.float32).reshape(4, C)]))
    em = np.zeros((4, P), np.float32)
    for j in range(4):
        em[j, HD * j:HD * (j + 1)] = 1.0
    common = {
        "y": y_full,
        "emat": em,
        "w4": w4,
        "w1": np.ascontiguousarray(np.asarray(W1, np.float32)),
        "w2": np.ascontiguousarray(np.asarray(W2, np.float32)),
        "ones32": np.ones((P, 1), np.float32),
        "bpack": bpack,
    }
    in_maps = []
    for c in range(NCORES):
        m = dict(common)
        m["xT"] = np.ascontiguousarray(xT_full[:, c * SH:(c + 1) * SH])
        in_maps.append(m)
    return in_maps


def kernel(lidar_features, image_features, Wq, bq, Wk, bk, Wv, bv, Wo, bo,
           g1, b1, W1, bf1, W2, bf2, g2, b2, num_heads, **run_kwargs):
    assert int(num_heads) == NH
    nc = _get_nc()
    in_maps = make_in_maps(lidar_features, image_features, Wq, bq, Wk, bk,
                           Wv, bv, Wo, bo, g1, b1, W1, bf1, W2, bf2, g2, b2)
    res = run_bass_kernel_spmd(nc, in_maps, core_ids=list(range(NCORES)),
                               **run_kwargs)
    full = np.concatenate([res.results[c]["out"] for c in range(NCORES)],
                          axis=1)
    kernel.last_results = res
    return full.reshape(1, C, 64, 64).astype(np.float32)


kernel.last_results = None
